# revision 1
# baseline (speedup 1.0000x reference)
"""Trainium2 Bass kernel for AttLayer pooling (B=32, T=2048, D=1024, H=5).

Math (equivalent to reference up to exact cancellation of the softmax
normalization): since |tanh| <= 1, scores s[b,t] are bounded by ||uw||_1, so
exp needs no max-subtraction, and the masked renormalization cancels the
softmax denominator:

    out[b,:] = sum_t x[b,t,:] * g[b,t] / sum_t g[b,t]
    g[b,t]   = exp(s[b,t] + masklog[b,t]),  masklog = 0 or -1e30
    s[b,t]   = sum_h tanh( (x @ W)[b,t,h] + fea[b,t]*Wf[h] + bw[h] ) * uw[h]

Everything is tile-local (no cross-T dependency), so the kernel streams x
in a single pass. Data-parallel across batch: 8 cores x 4 batches each.

Per 128-t tile of x [128, 1024]:
  - PE transposes x chunks (f32r, is_transpose) -> psum -> DVE copy -> xT sbuf
  - scores: psum[5, T_GRP] += W_chunk.T @ xT_chunk   (f32r, N=T_GRP)
            + fea part via K=1 matmul (lhsT=Wf [1,5], rhs=fea row)
  - ACT tanh(scores + bw) -> tanh_b rows 0..4; masklog precomputed in row 5
  - uw matmul per tile: lhsT = tanh_b[:, chunk] [6,128], rhs = uw_aug [6,1]
    -> s' column [128,1] in psum (mask fold: uw_aug[5]=1, row5 = masklog)
  - ACT exp -> g [128,1] f32r
  - num matmuls: psum[1, 1025] += g.T @ [x | ones]  (f32r; col 1024 = den)
Final per batch: out = num * reciprocal(den), DMA out.
"""

import sys

sys.path.insert(0, "/opt/trn_rl_repo")

import numpy as np

import concourse.bass as bass
import concourse.mybir as mybir
import concourse.tile as tile
from concourse import bacc
from concourse.masks import make_identity
from concourse import bass_isa

F32 = mybir.dt.float32
F32R = mybir.dt.float32r
BF16 = mybir.dt.bfloat16
U8 = mybir.dt.uint8
AF = mybir.ActivationFunctionType

P = 128          # partitions / t-tile size
D = 1024         # feature dim
H = 5            # attention hidden dim
NCHUNK = D // P  # 8 d-chunks per tile


def build_kernel(b_shard: int, T: int, t_grp: int = 512, dma_grp: int = 512):
    """Build the per-core Bass program.

    b_shard: batches per core; T: sequence length; t_grp: t per compute
    group (multiple of 128, <= 512); dma_grp: t per DMA chunk (multiple of
    t_grp).
    """
    assert t_grp % P == 0 and T % dma_grp == 0 and dma_grp % t_grp == 0
    jg = t_grp // P            # tiles per compute group
    jd = dma_grp // P          # tiles per DMA chunk
    n_dma = T // dma_grp
    grp_per_dma = dma_grp // t_grp

    nc = bacc.Bacc(None)

    x_temp = nc.dram_tensor("x_temp", [b_shard, T, D], F32R, kind="ExternalInput")
    x_fea = nc.dram_tensor("x_fea", [b_shard, T], F32R, kind="ExternalInput")
    mask = nc.dram_tensor("mask", [b_shard, T], U8, kind="ExternalInput")
    W_temp = nc.dram_tensor("W_temp", [D, H], F32, kind="ExternalInput")
    W_fea = nc.dram_tensor("W_fea", [1, H], F32R, kind="ExternalInput")
    bw = nc.dram_tensor("bw", [H], F32, kind="ExternalInput")
    uw = nc.dram_tensor("uw", [H], F32, kind="ExternalInput")
    out = nc.dram_tensor("out", [b_shard, D], F32, kind="ExternalOutput")

    with tile.TileContext(nc) as tc:
        with (
            tc.tile_pool(name="consts", bufs=1) as consts,
            tc.tile_pool(name="xpool", bufs=3) as xpool,
            tc.tile_pool(name="xtpool", bufs=3) as xtpool,
            tc.tile_pool(name="rows", bufs=2) as rows,
            tc.tile_pool(name="small", bufs=2) as small,
            tc.tile_pool(name="tp_ps", bufs=3, space="PSUM") as tp_ps,
            tc.tile_pool(name="sc_ps", bufs=2, space="PSUM") as sc_ps,
            tc.tile_pool(name="g_ps", bufs=1, space="PSUM") as g_ps,
            tc.tile_pool(name="num_ps", bufs=1, space="PSUM") as num_ps,
        ):
            # ---- constants ----
            # Transposes + scores matmuls run in bf16 (x cast on GpSimd);
            # num matmuls stay f32r on the raw DMA'd x (exact-ish).
            ident = consts.tile([P, P], BF16)
            make_identity(nc, ident[:])
            w_f = consts.tile([P, NCHUNK, H], F32)
            nc.sync.dma_start(w_f[:], W_temp.rearrange("(c p) h -> p c h", p=P))
            w_sb = consts.tile([P, NCHUNK, H], BF16)
            nc.vector.tensor_copy(w_sb[:], w_f[:])
            wf_sb = consts.tile([1, H], F32R)
            nc.sync.dma_start(wf_sb[:], W_fea[:])
            bw_sb = consts.tile([H, 1], F32)
            nc.sync.dma_start(bw_sb[:], bw[:, None])
            # uw_aug = [uw; 1.0]: memset whole tile to 1.0, DMA uw over rows 0..4
            # (engine ops cannot write at base partition 5, DMA can overwrite 0..4)
            uwa_f = consts.tile([H + 1, 2], F32)
            nc.vector.memset(uwa_f[:], 1.0)
            nc.sync.dma_start(uwa_f[:H, 0:1], uw[:, None])
            nc.sync.dma_start(uwa_f[:H, 1:2], uw[:, None])
            uwa_sb = consts.tile([H + 1, 2], F32R)
            nc.vector.tensor_copy(uwa_sb[:], uwa_f[:])

            for b in range(b_shard):
                # ---- per-batch rows ----
                fea_sb = rows.tile([1, T], F32R, tag="fea")
                nc.sync.dma_start(fea_sb[:], x_fea[b : b + 1, :])
                mask_f = rows.tile([1, T], F32, tag="maskf")
                nc.gpsimd.dma_start(mask_f[:], mask[b : b + 1, :])  # u8 -> f32 cast
                masklog = rows.tile([1, T], F32R, tag="masklog")
                nc.scalar.activation(
                    masklog[:], mask_f[:], AF.Copy, scale=1.0e30, bias=-1.0e30
                )
                # tanh_b rows 0..4 = tanh(scores) written per group;
                # row 5 = masklog = mask*1e30 - 1e30  (0 or -1e30).
                # SBUF->SBUF DMA: engines can't write at base partition 5.
                tanh_b = rows.tile([H + 1, T], F32R, tag="tanhb")
                nc.sync.dma_start(tanh_b[H : H + 1, :], masklog[:])

                nm = num_ps.tile([1, D], F32, tag="num")
                n_tiles = T // P
                g_sb = rows.tile([P, n_tiles], F32R, tag="gsb")

                # num matmuls for group g are emitted while group g+1's
                # scores run (one-group software pipeline), so the PE never
                # stalls on the exp(g) -> num-LDW dependency.
                pending = None

                def emit_num(p):
                    g_, x3_, gi_ = p
                    for j_ in range(jg):
                        tt_ = g_ * jg + j_
                        nc.tensor.matmul(
                            nm[:, 0:512],
                            g_sb[:, tt_ : tt_ + 1],
                            x3_[:, gi_ * jg + j_, 0:512],
                            start=(tt_ == 0),
                            stop=(tt_ == n_tiles - 1),
                        )
                        nc.tensor.matmul(
                            nm[:, 512:1024],
                            g_sb[:, tt_ : tt_ + 1],
                            x3_[:, gi_ * jg + j_, 512:1024],
                            start=(tt_ == 0),
                            stop=(tt_ == n_tiles - 1),
                        )

                for di in range(n_dma):
                    x3 = xpool.tile([P, jd, D], F32R, tag="x")
                    nc.sync.dma_start(
                        x3[:],
                        x_temp[b, di * dma_grp : (di + 1) * dma_grp, :].rearrange(
                            "(j p) d -> p j d", p=P
                        ),
                    )
                    for gi in range(grp_per_dma):
                        g = di * grp_per_dma + gi   # group index within batch
                        t0 = g * t_grp
                        # bf16 copy of this group's x for transposes/scores
                        xb = xtpool.tile([P, jg, D], BF16, tag="xb")
                        for j in range(jg):
                            src = x3[:, gi * jg + j, :].bitcast(F32)
                            if j % 4 == 3:
                                nc.scalar.copy(xb[:, j, :], src)
                            else:
                                nc.vector.tensor_copy(xb[:, j, :], src)
                        sc = sc_ps.tile([H, t_grp], F32, tag="sc")
                        # fea part: [5, t_grp] = Wf.T @ fea_row (K=1), starts accum
                        nc.tensor.matmul(
                            sc[:],
                            wf_sb[:],
                            fea_sb[:, t0 : t0 + t_grp],
                            start=True,
                            stop=False,
                        )
                        if pending is not None:
                            emit_num(pending)
                            pending = None
                        for cp in range(NCHUNK // 2):
                            tp = tp_ps.tile([P, 2, t_grp], BF16, tag="tp")
                            for c2 in range(2):
                                c = cp * 2 + c2
                                for j in range(jg):
                                    nc.tensor.transpose(
                                        tp[:, c2, j * P : (j + 1) * P],
                                        xb[:, j, c * P : (c + 1) * P],
                                        ident[:],
                                    )
                            xt = xtpool.tile([P, 2, t_grp], BF16, tag="xt")
                            nc.vector.tensor_copy(xt[:], tp[:])
                            for c2 in range(2):
                                c = cp * 2 + c2
                                nc.tensor.matmul(
                                    sc[:],
                                    w_sb[:, c, :],
                                    xt[:, c2, :],
                                    start=False,
                                    stop=(c == NCHUNK - 1),
                                )
                        # tanh(sc + bw) -> tanh_b rows 0..4
                        nc.scalar.activation(
                            tanh_b[:H, t0 : t0 + t_grp], sc[:], AF.Tanh, bias=bw_sb[:]
                        )
                        # uw matmuls: one [128,1] s' column per tile
                        gp = g_ps.tile([P, jg, 2], F32, tag="g")
                        for j in range(jg):
                            nc.tensor.matmul(
                                gp[:, j, :],
                                tanh_b[:, t0 + j * P : t0 + (j + 1) * P],
                                uwa_sb[:],
                                start=True,
                                stop=True,
                            )
                        nc.scalar.activation(
                            g_sb[:, g * jg : (g + 1) * jg], gp[:, :, 0], AF.Exp
                        )
                        pending = (g, x3, gi)

                if pending is not None:
                    emit_num(pending)
                    pending = None

                # den = sum of g: DVE free-reduce then GpSimd partition reduce
                gcs = small.tile([P, 1], F32, tag="gcs")
                nc.vector.tensor_reduce(
                    gcs[:],
                    g_sb[:].bitcast(F32),
                    axis=mybir.AxisListType.X,
                    op=mybir.AluOpType.add,
                )
                den_sb = small.tile([P, 1], F32, tag="densb")
                nc.gpsimd.partition_all_reduce(
                    den_sb[:], gcs[:], channels=P, reduce_op=bass_isa.ReduceOp.add
                )
                inv = small.tile([1, 1], F32, tag="inv")
                nc.vector.reciprocal(inv[:], den_sb[0:1, :])
                o_sb = small.tile([1, D], F32, tag="osb")
                nc.vector.tensor_scalar_mul(o_sb[:], nm[:, :D], inv[:])
                nc.sync.dma_start(out[b : b + 1, :], o_sb[:])

    nc.finalize()
    return nc


_NC_CACHE = {}


def _get_nc(b_shard, T):
    key = (b_shard, T)
    if key not in _NC_CACHE:
        _NC_CACHE[key] = build_kernel(b_shard, T)
    return _NC_CACHE[key]


def kernel(x_temp, x_fea, mask, W_temp, W_fea, bw, uw) -> np.ndarray:
    from concourse.bass_utils import run_bass_kernel_spmd

    B, T, D_ = x_temp.shape
    n_cores = 8
    assert B % n_cores == 0
    bs = B // n_cores

    nc = _get_nc(bs, T)

    x_temp = np.ascontiguousarray(x_temp, dtype=np.float32)
    x_fea = np.ascontiguousarray(x_fea, dtype=np.float32)
    mask_u8 = np.ascontiguousarray(mask).view(np.uint8)
    W_temp = np.ascontiguousarray(W_temp, dtype=np.float32)
    W_fea = np.ascontiguousarray(W_fea, dtype=np.float32)
    bw = np.ascontiguousarray(bw, dtype=np.float32)
    uw = np.ascontiguousarray(uw, dtype=np.float32)

    in_maps = []
    for i in range(n_cores):
        in_maps.append(
            {
                "x_temp": x_temp[i * bs : (i + 1) * bs],
                "x_fea": x_fea[i * bs : (i + 1) * bs],
                "mask": mask_u8[i * bs : (i + 1) * bs],
                "W_temp": W_temp,
                "W_fea": W_fea,
                "bw": bw,
                "uw": uw,
            }
        )

    res = run_bass_kernel_spmd(nc, in_maps, core_ids=list(range(n_cores)))
    return np.concatenate([r["out"] for r in res.results], axis=0)



# revision 6
# speedup vs baseline: 1.0120x; 1.0120x over previous
"""Trainium2 Bass kernel for AttLayer pooling (B=32, T=2048, D=1024, H=5).

Math (equivalent to reference up to exact cancellation of the softmax
normalization): since |tanh| <= 1, scores s[b,t] are bounded by ||uw||_1, so
exp needs no max-subtraction, and the masked renormalization cancels the
softmax denominator:

    out[b,:] = sum_t x[b,t,:] * g[b,t] / sum_t g[b,t]
    g[b,t]   = exp(s[b,t] + masklog[b,t]),  masklog = 0 or -1e30
    s[b,t]   = sum_h tanh( (x @ W)[b,t,h] + fea[b,t]*Wf[h] + bw[h] ) * uw[h]

Everything is tile-local (no cross-T dependency), so the kernel streams x
in a single pass. Data-parallel across batch: 8 cores x 4 batches each.

Per 128-t tile of x [128, 1024]:
  - PE transposes x chunks (f32r, is_transpose) -> psum -> DVE copy -> xT sbuf
  - scores: psum[5, T_GRP] += W_chunk.T @ xT_chunk   (f32r, N=T_GRP)
            + fea part via K=1 matmul (lhsT=Wf [1,5], rhs=fea row)
  - ACT tanh(scores + bw) -> tanh_b rows 0..4; masklog precomputed in row 5
  - uw matmul per tile: lhsT = tanh_b[:, chunk] [6,128], rhs = uw_aug [6,1]
    -> s' column [128,1] in psum (mask fold: uw_aug[5]=1, row5 = masklog)
  - ACT exp -> g [128,1] f32r
  - num matmuls: psum[1, 1025] += g.T @ [x | ones]  (f32r; col 1024 = den)
Final per batch: out = num * reciprocal(den), DMA out.
"""

import sys

sys.path.insert(0, "/opt/trn_rl_repo")

import numpy as np

import concourse.bass as bass
import concourse.mybir as mybir
import concourse.tile as tile
from concourse import bacc
from concourse.masks import make_identity
from concourse import bass_isa

F32 = mybir.dt.float32
F32R = mybir.dt.float32r
BF16 = mybir.dt.bfloat16
U8 = mybir.dt.uint8
AF = mybir.ActivationFunctionType

P = 128          # partitions / t-tile size
D = 1024         # feature dim
H = 5            # attention hidden dim
NCHUNK = D // P  # 8 d-chunks per tile


def build_kernel(b_shard: int, T: int, t_grp: int = 512, dma_grp: int = 512):
    """Build the per-core Bass program.

    b_shard: batches per core; T: sequence length; t_grp: t per compute
    group (multiple of 128, <= 512); dma_grp: t per DMA chunk (multiple of
    t_grp).
    """
    assert t_grp % P == 0 and T % dma_grp == 0 and dma_grp % t_grp == 0
    jg = t_grp // P            # tiles per compute group
    jd = dma_grp // P          # tiles per DMA chunk
    n_dma = T // dma_grp
    grp_per_dma = dma_grp // t_grp

    nc = bacc.Bacc(None)

    x_temp = nc.dram_tensor("x_temp", [b_shard, T, D], F32R, kind="ExternalInput")
    x_fea = nc.dram_tensor("x_fea", [b_shard, T], F32R, kind="ExternalInput")
    mask = nc.dram_tensor("mask", [b_shard, T], U8, kind="ExternalInput")
    W_temp = nc.dram_tensor("W_temp", [D, H], F32, kind="ExternalInput")
    W_fea = nc.dram_tensor("W_fea", [1, H], F32R, kind="ExternalInput")
    bw = nc.dram_tensor("bw", [H], F32, kind="ExternalInput")
    uw = nc.dram_tensor("uw", [H], F32, kind="ExternalInput")
    out = nc.dram_tensor("out", [b_shard, D], F32, kind="ExternalOutput")

    with tile.TileContext(nc) as tc:
        with (
            tc.tile_pool(name="consts", bufs=1) as consts,
            tc.tile_pool(name="xpool", bufs=4) as xpool,
            tc.tile_pool(name="xtpool", bufs=3) as xtpool,
            tc.tile_pool(name="rows", bufs=2) as rows,
            tc.tile_pool(name="small", bufs=2) as small,
            tc.tile_pool(name="tp_ps", bufs=3, space="PSUM") as tp_ps,
            tc.tile_pool(name="sc_ps", bufs=2, space="PSUM") as sc_ps,
            tc.tile_pool(name="g_ps", bufs=1, space="PSUM") as g_ps,
            tc.tile_pool(name="num_ps", bufs=1, space="PSUM") as num_ps,
        ):
            # ---- constants ----
            # Transposes + scores matmuls run in bf16 (x cast on GpSimd);
            # num matmuls stay f32r on the raw DMA'd x (exact-ish).
            ident = consts.tile([P, P], BF16)
            make_identity(nc, ident[:])
            w_f = consts.tile([P, NCHUNK, H], F32)
            nc.sync.dma_start(w_f[:], W_temp.rearrange("(c p) h -> p c h", p=P))
            w_sb = consts.tile([P, NCHUNK, H], BF16)
            nc.vector.tensor_copy(w_sb[:], w_f[:])
            wf_sb = consts.tile([1, H], F32R)
            nc.sync.dma_start(wf_sb[:], W_fea[:])
            bw_sb = consts.tile([H, 1], F32)
            nc.sync.dma_start(bw_sb[:], bw[:, None])
            # uw_aug = [uw; 1.0]: memset whole tile to 1.0, DMA uw over rows 0..4
            # (engine ops cannot write at base partition 5, DMA can overwrite 0..4)
            uwa_f = consts.tile([H + 1, 2], F32)
            nc.vector.memset(uwa_f[:], 1.0)
            nc.sync.dma_start(uwa_f[:H, 0:1], uw[:, None])
            nc.sync.dma_start(uwa_f[:H, 1:2], uw[:, None])
            uwa_sb = consts.tile([H + 1, 2], F32R)
            nc.vector.tensor_copy(uwa_sb[:], uwa_f[:])

            for b in range(b_shard):
                # ---- per-batch rows ----
                fea_sb = rows.tile([1, T], F32R, tag="fea")
                nc.sync.dma_start(fea_sb[:], x_fea[b : b + 1, :])
                mask_f = rows.tile([1, T], F32, tag="maskf")
                nc.gpsimd.dma_start(mask_f[:], mask[b : b + 1, :])  # u8 -> f32 cast
                masklog = rows.tile([1, T], F32R, tag="masklog")
                nc.scalar.activation(
                    masklog[:], mask_f[:], AF.Copy, scale=1.0e30, bias=-1.0e30
                )
                # tanh_b rows 0..4 = tanh(scores) written per group;
                # row 5 = masklog = mask*1e30 - 1e30  (0 or -1e30).
                # SBUF->SBUF DMA: engines can't write at base partition 5.
                tanh_b = rows.tile([H + 1, T], F32R, tag="tanhb")
                nc.sync.dma_start(tanh_b[H : H + 1, :], masklog[:])

                nm = num_ps.tile([1, D], F32, tag="num")
                n_tiles = T // P
                g_sb = rows.tile([P, n_tiles], F32R, tag="gsb")

                # uw/exp/num for group g are emitted while group g+1's
                # transposes/scores run (one-group software pipeline), so the
                # PE never stalls on ACT tanh/exp latencies.
                pend_uw = None   # (g,) -> emit uw matmuls + exp for group g
                pend_num = None  # (g, x3, gi) -> emit num matmuls for group g

                def emit_uw(p):
                    (g_,) = p
                    t0_ = g_ * t_grp
                    gp = g_ps.tile([P, jg, 2], F32, tag="g")
                    for j_ in range(jg):
                        nc.tensor.matmul(
                            gp[:, j_, :],
                            tanh_b[:, t0_ + j_ * P : t0_ + (j_ + 1) * P],
                            uwa_sb[:],
                            start=True,
                            stop=True,
                        )
                    nc.scalar.activation(
                        g_sb[:, g_ * jg : (g_ + 1) * jg], gp[:, :, 0], AF.Exp
                    )

                def emit_num(p):
                    g_, x3_, gi_ = p
                    for j_ in range(jg):
                        tt_ = g_ * jg + j_
                        nc.tensor.matmul(
                            nm[:, 0:512],
                            g_sb[:, tt_ : tt_ + 1],
                            x3_[:, gi_ * jg + j_, 0:512],
                            start=(tt_ == 0),
                            stop=(tt_ == n_tiles - 1),
                        )
                        nc.tensor.matmul(
                            nm[:, 512:1024],
                            g_sb[:, tt_ : tt_ + 1],
                            x3_[:, gi_ * jg + j_, 512:1024],
                            start=(tt_ == 0),
                            stop=(tt_ == n_tiles - 1),
                        )

                for di in range(n_dma):
                    x3 = xpool.tile([P, jd, D], F32R, tag="x")
                    nc.sync.dma_start(
                        x3[:],
                        x_temp[b, di * dma_grp : (di + 1) * dma_grp, :].rearrange(
                            "(j p) d -> p j d", p=P
                        ),
                    )
                    for gi in range(grp_per_dma):
                        g = di * grp_per_dma + gi   # group index within batch
                        t0 = g * t_grp
                        # bf16 copy of this group's x for transposes/scores
                        xb = xtpool.tile([P, jg, D], BF16, tag="xb")
                        for j in range(jg):
                            src = x3[:, gi * jg + j, :].bitcast(F32)
                            if j % 4 == 3:
                                nc.scalar.copy(xb[:, j, :], src)
                            else:
                                nc.vector.tensor_copy(xb[:, j, :], src)
                        sc = sc_ps.tile([H, t_grp], F32, tag="sc")
                        # fea part: [5, t_grp] = Wf.T @ fea_row (K=1), starts accum
                        nc.tensor.matmul(
                            sc[:],
                            wf_sb[:],
                            fea_sb[:, t0 : t0 + t_grp],
                            start=True,
                            stop=False,
                        )
                        # PE stream per group (software-pipelined): the score
                        # matmuls trail the transposes by one cp-pair so the
                        # DVE psum->sbuf copy is never on the PE critical path;
                        # uw/exp/num of the previous group fill the middle.
                        pend_mm = None

                        def emit_mm(cp_, xt_):
                            for c2_ in range(2):
                                c_ = cp_ * 2 + c2_
                                nc.tensor.matmul(
                                    sc[:],
                                    w_sb[:, c_, :],
                                    xt_[:, c2_, :],
                                    start=False,
                                    stop=(c_ == NCHUNK - 1),
                                )

                        for cp in range(NCHUNK // 2):
                            tp = tp_ps.tile([P, 2, t_grp], BF16, tag="tp")
                            for c2 in range(2):
                                c = cp * 2 + c2
                                for j in range(jg):
                                    nc.tensor.transpose(
                                        tp[:, c2, j * P : (j + 1) * P],
                                        xb[:, j, c * P : (c + 1) * P],
                                        ident[:],
                                    )
                            xt = xtpool.tile([P, 2, t_grp], BF16, tag="xt")
                            nc.vector.tensor_copy(xt[:], tp[:])
                            if cp == 1 and pend_uw is not None:
                                emit_uw(pend_uw)
                            if cp == 2 and pend_num is not None:
                                emit_num(pend_num)
                            if pend_mm is not None:
                                emit_mm(*pend_mm)
                            pend_mm = (cp, xt)
                        pend_uw = None
                        pend_num = None
                        emit_mm(*pend_mm)
                        # tanh(sc + bw) -> tanh_b rows 0..4
                        nc.scalar.activation(
                            tanh_b[:H, t0 : t0 + t_grp], sc[:], AF.Tanh, bias=bw_sb[:]
                        )
                        pend_uw = (g,)
                        pend_num = (g, x3, gi)

                emit_uw(pend_uw)
                pend_uw = None
                emit_num(pend_num)
                pend_num = None

                # free the num psum bank immediately so the next batch's num
                # matmuls don't wait on the slow den-reduce tail
                nm_sb = small.tile([1, D], F32, tag="nmsb")
                nc.vector.tensor_copy(nm_sb[:], nm[:, :D])

                # den = sum of g: DVE free-reduce then GpSimd partition reduce
                gcs = small.tile([P, 1], F32, tag="gcs")
                nc.vector.tensor_reduce(
                    gcs[:],
                    g_sb[:].bitcast(F32),
                    axis=mybir.AxisListType.X,
                    op=mybir.AluOpType.add,
                )
                den_sb = small.tile([P, 1], F32, tag="densb")
                nc.gpsimd.partition_all_reduce(
                    den_sb[:], gcs[:], channels=P, reduce_op=bass_isa.ReduceOp.add
                )
                inv = small.tile([1, 1], F32, tag="inv")
                nc.vector.reciprocal(inv[:], den_sb[0:1, :])
                o_sb = small.tile([1, D], F32, tag="osb")
                nc.vector.tensor_scalar_mul(o_sb[:], nm_sb[:], inv[:])
                nc.sync.dma_start(out[b : b + 1, :], o_sb[:])

    nc.finalize()
    return nc


_NC_CACHE = {}


def _get_nc(b_shard, T):
    key = (b_shard, T)
    if key not in _NC_CACHE:
        _NC_CACHE[key] = build_kernel(b_shard, T)
    return _NC_CACHE[key]


def kernel(x_temp, x_fea, mask, W_temp, W_fea, bw, uw) -> np.ndarray:
    from concourse.bass_utils import run_bass_kernel_spmd

    B, T, D_ = x_temp.shape
    n_cores = 8
    assert B % n_cores == 0
    bs = B // n_cores

    nc = _get_nc(bs, T)

    x_temp = np.ascontiguousarray(x_temp, dtype=np.float32)
    x_fea = np.ascontiguousarray(x_fea, dtype=np.float32)
    mask_u8 = np.ascontiguousarray(mask).view(np.uint8)
    W_temp = np.ascontiguousarray(W_temp, dtype=np.float32)
    W_fea = np.ascontiguousarray(W_fea, dtype=np.float32)
    bw = np.ascontiguousarray(bw, dtype=np.float32)
    uw = np.ascontiguousarray(uw, dtype=np.float32)

    in_maps = []
    for i in range(n_cores):
        in_maps.append(
            {
                "x_temp": x_temp[i * bs : (i + 1) * bs],
                "x_fea": x_fea[i * bs : (i + 1) * bs],
                "mask": mask_u8[i * bs : (i + 1) * bs],
                "W_temp": W_temp,
                "W_fea": W_fea,
                "bw": bw,
                "uw": uw,
            }
        )

    res = run_bass_kernel_spmd(nc, in_maps, core_ids=list(range(n_cores)))
    return np.concatenate([r["out"] for r in res.results], axis=0)



# revision 11
# speedup vs baseline: 1.1250x; 1.1116x over previous
"""Trainium2 Bass kernel for AttLayer pooling (B=32, T=2048, D=1024, H=5).

Math (equivalent to reference up to exact cancellation of the softmax
normalization): since |tanh| <= 1, scores s[b,t] are bounded by ||uw||_1, so
exp needs no max-subtraction, and the masked renormalization cancels the
softmax denominator:

    out[b,:] = sum_t x[b,t,:] * g[b,t] / sum_t g[b,t]
    g[b,t]   = exp(s[b,t] - 256) with s' = s + 256*mask  (mask fold: masked
               entries get exp(s-256) ~ 1e-110 ~ 0; unmasked are exact)
    s[b,t]   = sum_h tanh( (x @ W)[b,t,h] + fea[b,t]*Wf[h] + bw[h] ) * uw[h]

v2 design (engine budget per core, 4 batches):
  - x arrives as bf16 via GpSimd casting DMA (f32 HBM -> bf16 SBUF), so no
    cast ops on compute engines.  DMA ~94us (streaming floor) is the target
    critical path.
  - PE (~86us): transposes x (bf16, 77ns/128x128), scores GEMM W.T @ xT
    ([5,512] per chunk), fea K=1 matmul, uw matmul with M=1 (lhsT=[6,1])
    giving s' as a psum ROW [1,512], final [128,8] output transpose.
  - ACT (~58us): half the psum->sbuf xT copies, tanh, exp (bias=-256) with
    accum_out giving the per-group denominator for free.
  - DVE (~64us): other half of copies + the whole num reduction:
    scalar_tensor_tensor(xt * g_bcast, accum_out) per (chunk, group) -> num
    columns; num never touches the PE.
  - GpSimd (~30us): casting DMAs, mask u8->f32 DMA straight into tanh row 5,
    per-group partition_broadcast of g.
"""

import sys

sys.path.insert(0, "/opt/trn_rl_repo")

import numpy as np

import concourse.bass as bass
import concourse.mybir as mybir
import concourse.tile as tile
from concourse import bacc
from concourse.masks import make_identity

F32 = mybir.dt.float32
F32R = mybir.dt.float32r
BF16 = mybir.dt.bfloat16
U8 = mybir.dt.uint8
AF = mybir.ActivationFunctionType

P = 128          # partitions / t-tile size
D = 1024         # feature dim
H = 5            # attention hidden dim
NCHUNK = D // P  # 8 d-chunks
MC = 256.0       # mask fold constant: s' = s + MC*mask, exp bias -MC


def build_kernel(b_shard: int, T: int, t_grp: int = 512):
    assert T % t_grp == 0 and t_grp % P == 0
    jg = t_grp // P              # tiles per group (4)
    n_grp = T // t_grp           # groups per batch (4)

    nc = bacc.Bacc(None)

    x_temp = nc.dram_tensor("x_temp", [b_shard, T, D], F32, kind="ExternalInput")
    x_fea = nc.dram_tensor("x_fea", [b_shard, T], F32R, kind="ExternalInput")
    mask = nc.dram_tensor("mask", [b_shard, T], U8, kind="ExternalInput")
    W_temp = nc.dram_tensor("W_temp", [D, H], F32, kind="ExternalInput")
    W_fea = nc.dram_tensor("W_fea", [1, H], F32R, kind="ExternalInput")
    bw = nc.dram_tensor("bw", [H], F32, kind="ExternalInput")
    uw = nc.dram_tensor("uw", [H], F32, kind="ExternalInput")
    out = nc.dram_tensor("out", [b_shard, D], F32, kind="ExternalOutput")

    with tile.TileContext(nc) as tc:
        with (
            tc.tile_pool(name="consts", bufs=1) as consts,
            tc.tile_pool(name="xpool", bufs=4) as xpool,
            tc.tile_pool(name="xtpool", bufs=8) as xtpool,
            tc.tile_pool(name="gbc", bufs=3) as gbcp,
            tc.tile_pool(name="rows", bufs=2) as rows,
            tc.tile_pool(name="small", bufs=2) as small,
            tc.tile_pool(name="scr", bufs=1) as scr,
            tc.tile_pool(name="tp_ps", bufs=3, space="PSUM") as tp_ps,
            tc.tile_pool(name="sc_ps", bufs=2, space="PSUM") as sc_ps,
            tc.tile_pool(name="sp_ps", bufs=2, space="PSUM") as sp_ps,
            tc.tile_pool(name="ot_ps", bufs=1, space="PSUM") as ot_ps,
        ):
            # ---- constants ----
            ident = consts.tile([P, P], BF16)
            make_identity(nc, ident[:])
            identf = consts.tile([P, P], F32)
            make_identity(nc, identf[:])
            w_f = consts.tile([P, NCHUNK, H], F32)
            nc.sync.dma_start(w_f[:], W_temp.rearrange("(c p) h -> p c h", p=P))
            w_sb = consts.tile([P, NCHUNK, H], BF16)
            nc.vector.tensor_copy(w_sb[:], w_f[:])
            wf_sb = consts.tile([1, H], F32R)
            nc.sync.dma_start(wf_sb[:], W_fea[:])
            bw_sb = consts.tile([H, 1], F32)
            nc.sync.dma_start(bw_sb[:], bw[:, None])
            # uwa = [uw; MC]: memset to MC, DMA uw over rows 0..4
            uwa_f = consts.tile([H + 1, 1], F32)
            nc.vector.memset(uwa_f[:], MC)
            nc.sync.dma_start(uwa_f[:H, 0:1], uw[:, None])
            uwa_sb = consts.tile([H + 1, 1], F32R)
            nc.vector.tensor_copy(uwa_sb[:], uwa_f[:])
            negmc = consts.tile([1, 1], F32)
            nc.vector.memset(negmc[:], -MC)
            # DVE scratch for the stt num ops (dead output)
            stt_scr = scr.tile([P, t_grp], BF16)

            prev = None  # (b, g, xt_a, xt_b) pending uw/exp/bcast/num work

            def emit_tail_for(p):
                """uw matmul + exp + bcast + num-stt for a finished group."""
                b_, g_, xts = p
                t0_ = g_ * t_grp
                gi_ = g_
                tanh_b, g_row, g_acc, nacc = batch_rows[b_]
                sp = sp_ps.tile([1, t_grp], F32, tag="sp")
                nc.tensor.matmul(
                    sp[:],
                    uwa_sb[:],
                    tanh_b[:, t0_ : t0_ + t_grp],
                    start=True,
                    stop=True,
                )
                nc.scalar.activation(
                    g_row[:, t0_ : t0_ + t_grp],
                    sp[:],
                    AF.Exp,
                    bias=negmc[:],
                    accum_out=g_acc[:, gi_ : gi_ + 1],
                )
                g_bc = gbcp.tile([P, t_grp], BF16, tag="gbc")
                nc.gpsimd.partition_broadcast(g_bc[:], g_row[:, t0_ : t0_ + t_grp])
                for half in range(2):
                    xt = xts[half]
                    for c2 in range(2):
                        c = half * 2 + c2
                        nc.vector.scalar_tensor_tensor(
                            stt_scr[:],
                            xt[:, c2, :],
                            1.0,
                            g_bc[:],
                            op0=mybir.AluOpType.mult,
                            op1=mybir.AluOpType.mult,
                            accum_out=nacc[:, c, gi_ : gi_ + 1],
                        )
                for half in range(2, 4):
                    xt = xts[half]
                    for c2 in range(2):
                        c = half * 2 + c2
                        nc.vector.scalar_tensor_tensor(
                            stt_scr[:],
                            xt[:, c2, :],
                            1.0,
                            g_bc[:],
                            op0=mybir.AluOpType.mult,
                            op1=mybir.AluOpType.mult,
                            accum_out=nacc[:, c, gi_ : gi_ + 1],
                        )

            batch_rows = {}

            for b in range(b_shard):
                # ---- per-batch rows ----
                fea_sb = rows.tile([1, T], F32R, tag="fea")
                nc.sync.dma_start(fea_sb[:], x_fea[b : b + 1, :])
                # tanh_b rows 0..4 = tanh(scores); row 5 = mask (0/1 f32),
                # folded into s' via uwa[5]=MC and exp bias=-MC.
                tanh_b = rows.tile([H + 1, T], F32R, tag="tanhb")
                nc.gpsimd.dma_start(tanh_b[H : H + 1, :], mask[b : b + 1, :])
                g_row = rows.tile([1, T], BF16, tag="grow")
                g_acc = rows.tile([1, n_grp], F32, tag="gacc")
                nacc = rows.tile([P, NCHUNK, n_grp], F32, tag="nacc")
                batch_rows[b] = (tanh_b, g_row, g_acc, nacc)

                for g in range(n_grp):
                    t0 = g * t_grp
                    # casting DMA: f32 HBM -> bf16 SBUF, one group (512 t)
                    x3 = xpool.tile([P, jg, D], BF16, tag="x")
                    nc.gpsimd.dma_start(
                        x3[:],
                        x_temp[b, t0 : t0 + t_grp, :].rearrange(
                            "(j p) d -> p j d", p=P
                        ),
                    )
                    sc = sc_ps.tile([H, t_grp], F32, tag="sc")
                    nc.tensor.matmul(
                        sc[:],
                        wf_sb[:],
                        fea_sb[:, t0 : t0 + t_grp],
                        start=True,
                        stop=False,
                    )
                    xts = []
                    for half in range(4):
                        tp = tp_ps.tile([P, 2, t_grp], BF16, tag="tp")
                        for c2 in range(2):
                            c = half * 2 + c2
                            for j in range(jg):
                                nc.tensor.transpose(
                                    tp[:, c2, j * P : (j + 1) * P],
                                    x3[:, j, c * P : (c + 1) * P],
                                    ident[:],
                                )
                        xt = xtpool.tile([P, 2, t_grp], BF16, tag="xt")
                        if half % 2 == 0:
                            nc.vector.tensor_copy(xt[:], tp[:])
                        else:
                            nc.scalar.copy(xt[:], tp[:])
                        xts.append(xt)
                    if prev is not None:
                        emit_tail_for(prev)
                        prev = None
                    for half in range(4):
                        for c2 in range(2):
                            c = half * 2 + c2
                            nc.tensor.matmul(
                                sc[:],
                                w_sb[:, c, :],
                                xts[half][:, c2, :],
                                start=False,
                                stop=(c == NCHUNK - 1),
                            )
                    nc.scalar.activation(
                        tanh_b[:H, t0 : t0 + t_grp], sc[:], AF.Tanh, bias=bw_sb[:]
                    )
                    prev = (b, g, xts)

                # flush the last group's tail at batch end
                emit_tail_for(prev)
                prev = None

                # ---- batch tail: num8, den, inv, output ----
                num8 = small.tile([P, NCHUNK], F32, tag="num8")
                nc.vector.tensor_reduce(
                    num8[:],
                    nacc[:],
                    axis=mybir.AxisListType.X,
                    op=mybir.AluOpType.add,
                )
                den = small.tile([1, 1], F32, tag="den")
                nc.vector.tensor_reduce(
                    den[:], g_acc[:], axis=mybir.AxisListType.X, op=mybir.AluOpType.add
                )
                inv = small.tile([1, 1], F32, tag="inv")
                nc.vector.reciprocal(inv[:], den[:])
                inv8 = small.tile([NCHUNK, 1], F32, tag="inv8")
                nc.gpsimd.partition_broadcast(inv8[:], inv[:], channels=NCHUNK)
                ot = ot_ps.tile([NCHUNK, P], F32, tag="ot")
                nc.tensor.transpose(ot[:], num8[:], identf[:])
                o_sb = small.tile([NCHUNK, P], F32, tag="osb")
                nc.scalar.activation(
                    o_sb[:], ot[:], AF.Copy, scale=inv8[:]
                )
                nc.sync.dma_start(
                    out[b : b + 1, :].rearrange("o (c p) -> (o c) p", p=P), o_sb[:]
                )

    nc.finalize()
    return nc


_NC_CACHE = {}


def _get_nc(b_shard, T):
    key = (b_shard, T)
    if key not in _NC_CACHE:
        _NC_CACHE[key] = build_kernel(b_shard, T)
    return _NC_CACHE[key]


def kernel(x_temp, x_fea, mask, W_temp, W_fea, bw, uw) -> np.ndarray:
    from concourse.bass_utils import run_bass_kernel_spmd

    B, T, D_ = x_temp.shape
    n_cores = 8
    assert B % n_cores == 0
    bs = B // n_cores

    nc = _get_nc(bs, T)

    x_temp = np.ascontiguousarray(x_temp, dtype=np.float32)
    x_fea = np.ascontiguousarray(x_fea, dtype=np.float32)
    mask_u8 = np.ascontiguousarray(mask).view(np.uint8)
    W_temp = np.ascontiguousarray(W_temp, dtype=np.float32)
    W_fea = np.ascontiguousarray(W_fea, dtype=np.float32)
    bw = np.ascontiguousarray(bw, dtype=np.float32)
    uw = np.ascontiguousarray(uw, dtype=np.float32)

    in_maps = []
    for i in range(n_cores):
        in_maps.append(
            {
                "x_temp": x_temp[i * bs : (i + 1) * bs],
                "x_fea": x_fea[i * bs : (i + 1) * bs],
                "mask": mask_u8[i * bs : (i + 1) * bs],
                "W_temp": W_temp,
                "W_fea": W_fea,
                "bw": bw,
                "uw": uw,
            }
        )

    res = run_bass_kernel_spmd(nc, in_maps, core_ids=list(range(n_cores)))
    return np.concatenate([r["out"] for r in res.results], axis=0)


# revision 14
# speedup vs baseline: 1.2473x; 1.1087x over previous
"""Trainium2 Bass kernel for AttLayer pooling (B=32, T=2048, D=1024, H=5).

Math (equivalent to reference up to exact cancellation of the softmax
normalization): since |tanh| <= 1, scores s[b,t] are bounded by ||uw||_1, so
exp needs no max-subtraction, and the masked renormalization cancels the
softmax denominator:

    out[b,:] = sum_t x[b,t,:] * g[b,t] / sum_t g[b,t]
    g[b,t]   = exp(s[b,t] - 256) with s' = s + 256*mask  (mask fold: masked
               entries get exp(s-256) ~ 1e-110 ~ 0; unmasked are exact)
    s[b,t]   = sum_h tanh( (x @ W)[b,t,h] + fea[b,t]*Wf[h] + bw[h] ) * uw[h]

v2 design (engine budget per core, 4 batches):
  - x arrives as bf16 via GpSimd casting DMA (f32 HBM -> bf16 SBUF), so no
    cast ops on compute engines.  DMA ~94us (streaming floor) is the target
    critical path.
  - PE (~86us): transposes x (bf16, 77ns/128x128), scores GEMM W.T @ xT
    ([5,512] per chunk), fea K=1 matmul, uw matmul with M=1 (lhsT=[6,1])
    giving s' as a psum ROW [1,512], final [128,8] output transpose.
  - ACT (~58us): half the psum->sbuf xT copies, tanh, exp (bias=-256) with
    accum_out giving the per-group denominator for free.
  - DVE (~64us): other half of copies + the whole num reduction:
    scalar_tensor_tensor(xt * g_bcast, accum_out) per (chunk, group) -> num
    columns; num never touches the PE.
  - GpSimd (~30us): casting DMAs, mask u8->f32 DMA straight into tanh row 5,
    per-group partition_broadcast of g.
"""

import sys

sys.path.insert(0, "/opt/trn_rl_repo")

import numpy as np

import concourse.bass as bass
import concourse.mybir as mybir
import concourse.tile as tile
from concourse import bacc
from concourse.masks import make_identity

F32 = mybir.dt.float32
F32R = mybir.dt.float32r
BF16 = mybir.dt.bfloat16
U8 = mybir.dt.uint8
AF = mybir.ActivationFunctionType

P = 128          # partitions / t-tile size
D = 1024         # feature dim
H = 5            # attention hidden dim
NCHUNK = D // P  # 8 d-chunks
MC = 256.0       # mask fold constant: s' = s + MC*mask, exp bias -MC


def build_kernel(b_shard: int, T: int, t_grp: int = 512):
    assert T % t_grp == 0 and t_grp % P == 0
    jg = t_grp // P              # tiles per group (4)
    n_grp = T // t_grp           # groups per batch (4)

    nc = bacc.Bacc(None)

    x_temp = nc.dram_tensor("x_temp", [b_shard, T, D], F32, kind="ExternalInput")
    x_fea = nc.dram_tensor("x_fea", [b_shard, T], F32R, kind="ExternalInput")
    mask = nc.dram_tensor("mask", [b_shard, T], U8, kind="ExternalInput")
    W_temp = nc.dram_tensor("W_temp", [D, H], F32, kind="ExternalInput")
    W_fea = nc.dram_tensor("W_fea", [1, H], F32R, kind="ExternalInput")
    bw = nc.dram_tensor("bw", [H], F32, kind="ExternalInput")
    uw = nc.dram_tensor("uw", [H], F32, kind="ExternalInput")
    out = nc.dram_tensor("out", [b_shard, D], F32, kind="ExternalOutput")

    with tile.TileContext(nc) as tc:
        with (
            tc.tile_pool(name="consts", bufs=1) as consts,
            tc.tile_pool(name="xpool", bufs=4) as xpool,
            tc.tile_pool(name="xtpool", bufs=8) as xtpool,
            tc.tile_pool(name="gbc", bufs=3) as gbcp,
            tc.tile_pool(name="rows", bufs=2) as rows,
            tc.tile_pool(name="small", bufs=2) as small,
            tc.tile_pool(name="scr", bufs=1) as scr,
            tc.tile_pool(name="tp_ps", bufs=3, space="PSUM") as tp_ps,
            tc.tile_pool(name="sc_ps", bufs=2, space="PSUM") as sc_ps,
            tc.tile_pool(name="sp_ps", bufs=2, space="PSUM") as sp_ps,
            tc.tile_pool(name="ot_ps", bufs=1, space="PSUM") as ot_ps,
        ):
            # ---- constants ----
            ident = consts.tile([P, P], BF16)
            make_identity(nc, ident[:])
            identf = consts.tile([P, P], F32)
            make_identity(nc, identf[:])
            w_f = consts.tile([P, NCHUNK, H], F32)
            nc.sync.dma_start(w_f[:], W_temp.rearrange("(c p) h -> p c h", p=P))
            w_sb = consts.tile([P, NCHUNK, H], BF16)
            nc.vector.tensor_copy(w_sb[:], w_f[:])
            wf_sb = consts.tile([1, H], F32R)
            nc.sync.dma_start(wf_sb[:], W_fea[:])
            bw_sb = consts.tile([H, 1], F32)
            nc.sync.dma_start(bw_sb[:], bw[:, None])
            # uwa = [uw; MC]: memset to MC, DMA uw over rows 0..4
            uwa_f = consts.tile([H + 1, 1], F32)
            nc.vector.memset(uwa_f[:], MC)
            nc.sync.dma_start(uwa_f[:H, 0:1], uw[:, None])
            uwa_sb = consts.tile([H + 1, 1], F32R)
            nc.vector.tensor_copy(uwa_sb[:], uwa_f[:])
            negmc = consts.tile([1, 1], F32)
            nc.vector.memset(negmc[:], -MC)
            # DVE scratch for the stt num ops (dead output)
            stt_scr = scr.tile([P, t_grp], BF16)

            prev = None  # (b, g, xt_a, xt_b) pending uw/exp/bcast/num work

            def emit_tail_for(p):
                """uw matmul + exp + bcast + num-stt for a finished group."""
                b_, g_, xts = p
                t0_ = g_ * t_grp
                gi_ = g_
                tanh_b, g_row, g_acc, nacc = batch_rows[b_]
                sp = sp_ps.tile([1, t_grp], F32, tag="sp")
                nc.tensor.matmul(
                    sp[:],
                    uwa_sb[:],
                    tanh_b[:, t0_ : t0_ + t_grp],
                    start=True,
                    stop=True,
                )
                nc.scalar.activation(
                    g_row[:, t0_ : t0_ + t_grp],
                    sp[:],
                    AF.Exp,
                    bias=negmc[:],
                    accum_out=g_acc[:, gi_ : gi_ + 1],
                )
                g_bc = gbcp.tile([P, t_grp], BF16, tag="gbc")
                nc.gpsimd.partition_broadcast(g_bc[:], g_row[:, t0_ : t0_ + t_grp])
                for half in range(4):
                    xt = xts[half]
                    for c2 in range(2):
                        c = half * 2 + c2
                        nc.vector.scalar_tensor_tensor(
                            stt_scr[:],
                            xt[:, c2, :],
                            1.0,
                            g_bc[:],
                            op0=mybir.AluOpType.mult,
                            op1=mybir.AluOpType.mult,
                            accum_out=nacc[:, c, gi_ : gi_ + 1],
                        )

            batch_rows = {}

            for b in range(b_shard):
                # ---- per-batch rows (x DMA of group 0 goes first so the
                # pipeline's head isn't waiting behind the mask DMA) ----
                fea_sb = rows.tile([1, T], F32R, tag="fea")
                tanh_b = rows.tile([H + 1, T], F32R, tag="tanhb")
                g_row = rows.tile([1, T], BF16, tag="grow")
                g_acc = rows.tile([1, n_grp], F32, tag="gacc")
                nacc = rows.tile([P, NCHUNK, n_grp], F32, tag="nacc")
                batch_rows[b] = (tanh_b, g_row, g_acc, nacc)

                for g in range(n_grp):
                    t0 = g * t_grp
                    # casting DMA: f32 HBM -> bf16 SBUF, one group (512 t)
                    x3 = xpool.tile([P, jg, D], BF16, tag="x")
                    nc.gpsimd.dma_start(
                        x3[:],
                        x_temp[b, t0 : t0 + t_grp, :].rearrange(
                            "(j p) d -> p j d", p=P
                        ),
                    )
                    if g == 0:
                        nc.sync.dma_start(fea_sb[:], x_fea[b : b + 1, :])
                        # tanh_b rows 0..4 = tanh(scores); row 5 = mask (0/1
                        # f32), folded via uwa[5]=MC and exp bias=-MC.
                        nc.gpsimd.dma_start(
                            tanh_b[H : H + 1, :], mask[b : b + 1, :]
                        )
                    sc = sc_ps.tile([H, t_grp], F32, tag="sc")
                    nc.tensor.matmul(
                        sc[:],
                        wf_sb[:],
                        fea_sb[:, t0 : t0 + t_grp],
                        start=True,
                        stop=False,
                    )
                    xts = []
                    for half in range(4):
                        tp = tp_ps.tile([P, 2, t_grp], BF16, tag="tp")
                        for c2 in range(2):
                            c = half * 2 + c2
                            for j in range(jg):
                                nc.tensor.transpose(
                                    tp[:, c2, j * P : (j + 1) * P],
                                    x3[:, j, c * P : (c + 1) * P],
                                    ident[:],
                                )
                        xt = xtpool.tile([P, 2, t_grp], BF16, tag="xt")
                        # bf16 pairs viewed as f32 halve the ap length
                        nc.scalar.copy(xt[:].bitcast(F32), tp[:].bitcast(F32))
                        xts.append(xt)
                    if prev is not None:
                        emit_tail_for(prev)
                        prev = None
                    for half in range(4):
                        for c2 in range(2):
                            c = half * 2 + c2
                            nc.tensor.matmul(
                                sc[:],
                                w_sb[:, c, :],
                                xts[half][:, c2, :],
                                start=False,
                                stop=(c == NCHUNK - 1),
                            )
                    nc.scalar.activation(
                        tanh_b[:H, t0 : t0 + t_grp], sc[:], AF.Tanh, bias=bw_sb[:]
                    )
                    prev = (b, g, xts)

                # flush the last group's tail at batch end
                emit_tail_for(prev)
                prev = None

                # ---- batch tail: num8, den, inv, output ----
                num8 = small.tile([P, NCHUNK], F32, tag="num8")
                nc.vector.tensor_reduce(
                    num8[:],
                    nacc[:],
                    axis=mybir.AxisListType.X,
                    op=mybir.AluOpType.add,
                )
                den = small.tile([1, 1], F32, tag="den")
                nc.vector.tensor_reduce(
                    den[:], g_acc[:], axis=mybir.AxisListType.X, op=mybir.AluOpType.add
                )
                inv = small.tile([1, 1], F32, tag="inv")
                nc.vector.reciprocal(inv[:], den[:])
                inv8 = small.tile([NCHUNK, 1], F32, tag="inv8")
                nc.gpsimd.partition_broadcast(inv8[:], inv[:], channels=NCHUNK)
                ot = ot_ps.tile([NCHUNK, P], F32, tag="ot")
                nc.tensor.transpose(ot[:], num8[:], identf[:])
                o_sb = small.tile([NCHUNK, P], F32, tag="osb")
                nc.scalar.activation(
                    o_sb[:], ot[:], AF.Copy, scale=inv8[:]
                )
                nc.sync.dma_start(
                    out[b : b + 1, :].rearrange("o (c p) -> (o c) p", p=P), o_sb[:]
                )

    nc.finalize()
    return nc


_NC_CACHE = {}


def _get_nc(b_shard, T):
    key = (b_shard, T)
    if key not in _NC_CACHE:
        _NC_CACHE[key] = build_kernel(b_shard, T)
    return _NC_CACHE[key]


def kernel(x_temp, x_fea, mask, W_temp, W_fea, bw, uw) -> np.ndarray:
    from concourse.bass_utils import run_bass_kernel_spmd

    B, T, D_ = x_temp.shape
    n_cores = 8
    assert B % n_cores == 0
    bs = B // n_cores

    nc = _get_nc(bs, T)

    x_temp = np.ascontiguousarray(x_temp, dtype=np.float32)
    x_fea = np.ascontiguousarray(x_fea, dtype=np.float32)
    mask_u8 = np.ascontiguousarray(mask).view(np.uint8)
    W_temp = np.ascontiguousarray(W_temp, dtype=np.float32)
    W_fea = np.ascontiguousarray(W_fea, dtype=np.float32)
    bw = np.ascontiguousarray(bw, dtype=np.float32)
    uw = np.ascontiguousarray(uw, dtype=np.float32)

    in_maps = []
    for i in range(n_cores):
        in_maps.append(
            {
                "x_temp": x_temp[i * bs : (i + 1) * bs],
                "x_fea": x_fea[i * bs : (i + 1) * bs],
                "mask": mask_u8[i * bs : (i + 1) * bs],
                "W_temp": W_temp,
                "W_fea": W_fea,
                "bw": bw,
                "uw": uw,
            }
        )

    res = run_bass_kernel_spmd(nc, in_maps, core_ids=list(range(n_cores)))
    return np.concatenate([r["out"] for r in res.results], axis=0)
